# revision 25
# baseline (speedup 1.0000x reference)
"""Trainium2 Bass kernel for nn_Conv2d (B=32, 256->256, 56x56, 3x3, pad=1) + bias.

Strategy
--------
Data-parallel over batch: 4 images per NeuronCore x 8 cores; weights/bias
replicated; no collectives.

Per core, the conv uses 1-D Winograd F(2,3) along W: each pair of output
columns (2tx, 2tx+1) is produced from 4 "points" p, where
M_p[h,tx] = sum_{kh,cin} Wt[p,kh,cin,cout] * V_p[cin, h+kh, tx] and
  out[h,2tx]   = M0 + M1 + M2 + bias
  out[h,2tx+1] = M1 - M2 - M3 + bias
This cuts PE rows streamed 1.5x vs direct shifted-matmul (4 point-streams of
28 tiles vs 3 kw-taps of 56+2 columns).  The input transform V (4 add/subs
over even/odd column phases, host-pre-split for unit stride) runs on the
Vector engine, drip-fed 2 ops per matmul group one image ahead (emitting it
as a block parks the in-order DVE queue behind WAR deps and stalls the PSUM
evictions).  The output transform + bias is fused into PSUM eviction:
DVE ops may read only ONE PSUM operand, so ACT evicts M2 and DVE chains
scalar_tensor_tensor ops.  Weights are host-transformed (G @ w along kw,
exact halves) and everything the PE touches is fp16 (1 cycle/row like f32r,
plus fast weight load, half DMA); accumulation is f32 in PSUM.  Weight DMAs
must have contiguous SBUF destinations (a strided dest is chopped into 256B
descriptors, ~5x slower) and the first-needed half is split across two idle
queues.  Measured rel err ~3.6e-4 (gate 2e-2); HW time ~155us vs 249us for
the direct f32r shifted-matmul baseline.
"""

import numpy as np

import concourse.bacc as bacc
import concourse.tile as tile
import concourse.mybir as mybir
from concourse.bass_utils import run_bass_kernel_spmd

F32 = mybir.dt.float32
F16 = mybir.dt.float16
ALU = mybir.AluOpType

B, CIN, COUT, H, W, K = 32, 256, 256, 56, 56, 3
NCORES = 8
BPC = B // NCORES          # images per core
HP = H + 2                 # padded rows (1 top, 1 bottom)
NT = 28                    # winograd tiles along W (2 output cols each)
XF = HP * 2 * 29           # x elems per (img, cinc): rows x (even|odd) x 29
SROWS = 14                 # output rows per PSUM chunk
NFREE = SROWS * NT         # 392 free positions per matmul
NS = H // SROWS            # 4 chunks per (img, cc)

_CACHE = {}


def _build():
    if "nc" in _CACHE:
        return _CACHE["nc"]
    nc = bacc.Bacc("TRN2", target_bir_lowering=False, debug=False,
                   num_swdge_queues=4)
    x_d = nc.dram_tensor("x", [BPC, 2, 128, XF], F16, kind="ExternalInput").ap()
    # w pre-arranged on host as [cc][cin128][p*kh*cinc*cout128] so TWO dmas
    # fill the SBUF weight tile, cc=0 half first (SWDGE triggers cost ~1us)
    w_d = nc.dram_tensor("w", [2, 128, 4 * K * 2 * 128], F16, kind="ExternalInput").ap()
    b_d = nc.dram_tensor("b", [128, 2], F32, kind="ExternalInput").ap()
    # device layout [img, cout, h, par, tx]; host interleaves par/tx -> w
    o_d = nc.dram_tensor("o", [BPC, COUT, H * W], F32, kind="ExternalOutput").ap()

    with tile.TileContext(nc) as tc:
        with (
            tc.tile_pool(name="wp", bufs=1) as wp,
            tc.tile_pool(name="xp", bufs=6) as xp,
            tc.tile_pool(name="vp", bufs=6) as vp,
            tc.tile_pool(name="op", bufs=3) as op,
            tc.tile_pool(name="sp", bufs=6) as sp,
            tc.tile_pool(name="pp", bufs=8, space="PSUM") as pp,
        ):
            eng = [nc.sync, nc.scalar]

            bias_t = wp.tile([128, 2], F32)
            # cc-major so each dma dest is contiguous (strided dest chops the
            # transfer into 256B descriptors -> descriptor-rate-bound, ~13us)
            w_t = wp.tile([128, 2, 4, K, 2, 128], F16)

            # HAM warmup: the PE sits idle until weights/V land (~12us), so
            # its clock gate re-throttles to 1.2GHz and the first ~3.4us of
            # real matmuls run at half speed.  Dummy matmuls on a zeroed
            # scratch tile keep the PE busy through the startup window.
            warm = wp.tile([128, NFREE], F16)
            nc.gpsimd.memset(warm, 0)
            wps = pp.tile([128, NFREE], F32, tag="ps")
            for _ in range(14):
                nc.tensor.matmul(wps, warm[:, 0:128], warm,
                                 start=True, stop=True)

            # weights + bias off the sync/scalar HWDGE queues (those carry x/
            # out).  cc=0 gates the first matmul group, so its two p-halves
            # ride different queues (gpsimd SWDGE + the idle vector HWDGE)
            # and land in ~half the single-queue transfer time.
            # cc0 in 4 p-chunks (the matmul stream consumes weights p-major,
            # so p=0 landing early unblocks the first group); cc1 in halves
            # so it lands before the first cc=1 group (~20us in)
            PW = K * 2 * 128
            for p in range(4):
                nc.gpsimd.dma_start(out=w_t[:, 0, p],
                                    in_=w_d[0][:, p * PW:(p + 1) * PW])
            nc.gpsimd.dma_start(out=w_t[:, 1, 0:2], in_=w_d[1][:, 0:2 * PW])
            nc.gpsimd.dma_start(out=w_t[:, 1, 2:4], in_=w_d[1][:, 2 * PW:])
            nc.gpsimd.dma_start(out=bias_t[:, :], in_=b_d[:, :])

            # x row-chunks: finer for img0 (gates PE start), coarse after
            RC0 = [(0, 16), (16, 30), (30, HP)]
            RC = [(0, 30), (30, HP)]

            def load_x(img):
                xs = []
                for ci in range(2):
                    x_t = xp.tile([128, HP, 2, 29], F16, tag="x")
                    for r0, r1 in (RC0 if img == 0 else RC):
                        eng[ci].dma_start(
                            out=x_t[:, r0:r1],
                            in_=x_d[img, ci, :, r0 * 58:r1 * 58],
                        )
                    xs.append(x_t)
                return xs

            def v_transform(xs, img):
                """Returns (v tiles, list of thunks each emitting one DVE op).
                Thunks are drained a couple per matmul group so the in-order
                DVE queue never parks a block of V ops in front of the
                PSUM-eviction ops."""
                vs = []
                for ci in range(2):
                    v_t = vp.tile([128, 4, HP, NT], F16, tag="v")
                    vs.append(v_t)
                ops = []
                for r0, r1 in (RC0 if img == 0 else RC):
                    for ci in range(2):
                        ev0 = xs[ci][:, r0:r1, 0, 0:28]
                        ev1 = xs[ci][:, r0:r1, 0, 1:29]
                        od0 = xs[ci][:, r0:r1, 1, 0:28]
                        od1 = xs[ci][:, r0:r1, 1, 1:29]
                        v = vs[ci]
                        ops += [
                            lambda v=v, a=ev0, b=ev1, r0=r0, r1=r1:
                                nc.vector.tensor_sub(v[:, 0, r0:r1], a, b),
                            lambda v=v, a=od0, b=ev1, r0=r0, r1=r1:
                                nc.vector.tensor_add(v[:, 1, r0:r1], a, b),
                            lambda v=v, a=ev1, b=od0, r0=r0, r1=r1:
                                nc.vector.tensor_sub(v[:, 2, r0:r1], a, b),
                            lambda v=v, a=od0, b=od1, r0=r0, r1=r1:
                                nc.vector.tensor_sub(v[:, 3, r0:r1], a, b),
                        ]
                return vs, ops

            def do_group(vs, cc, r_lo, r_hi, o_t):
                nf = (r_hi - r_lo) * NT
                ms = []
                for p in range(4):
                    m = pp.tile([128, NFREE], F32, tag="ps")
                    for mi, (kh, ci) in enumerate(
                        [(kh, ci) for kh in range(K) for ci in range(2)]
                    ):
                        nc.tensor.matmul(
                            m[:, :nf],
                            w_t[:, cc, p, kh, ci],
                            vs[ci][:, p, r_lo + kh:r_hi + kh],
                            start=(mi == 0),
                            stop=(mi == 5),
                        )
                    ms.append(m)
                # DVE ops may read at most ONE PSUM input each, so M2 is
                # evicted by the otherwise-idle ACT engine first.
                bias_ap = bias_t[:, cc:cc + 1]
                tm2 = sp.tile([128, NFREE], F32, tag="t")
                sa = sp.tile([128, NFREE], F32, tag="t")
                sb = sp.tile([128, NFREE], F32, tag="t")
                nc.scalar.copy(tm2[:, :nf], ms[2][:, :nf])
                nc.vector.scalar_tensor_tensor(          # M1 + b + M2
                    sa[:, :nf], ms[1][:, :nf], bias_ap, tm2[:, :nf],
                    ALU.add, ALU.add)
                nc.vector.tensor_add(                    # out0 = M0 + sa
                    o_t[:, r_lo:r_hi, 0], ms[0][:, :nf], sa[:, :nf])
                nc.vector.scalar_tensor_tensor(          # sa - 2*M2 = M1-M2+b
                    sb[:, :nf], tm2[:, :nf], -2.0, sa[:, :nf],
                    ALU.mult, ALU.add)
                nc.vector.tensor_sub(                    # out1 = sb - M3
                    o_t[:, r_lo:r_hi, 1], sb[:, :nf], ms[3][:, :nf])

            xs_all = [load_x(0), load_x(1)]
            vs_next, vops = v_transform(xs_all[0], 0)
            for f in vops:           # img0's V gates PE start: emit eagerly
                f()
            vops = []
            for img in range(BPC):
                vs_cur = vs_next
                if img + 1 < BPC:
                    if img + 2 < BPC:
                        xs_all.append(load_x(img + 2))
                    vs_next, vops = v_transform(xs_all[img + 1], img + 1)
                for cc in range(2):
                    o_t = op.tile([128, H, 2, NT], F32, tag="o")
                    # the very last chunk gates the fixed end-of-kernel
                    # barrier: halve it so its out-chain + store are short
                    chunks = [(0, 14), (14, 28), (28, 42), (42, 56)]
                    if img == BPC - 1 and cc == 1:
                        chunks = chunks[:3] + [(42, 49), (49, 56)]
                    for si, (r_lo, r_hi) in enumerate(chunks):
                        do_group(vs_cur, cc, r_lo, r_hi, o_t)
                        # store each chunk as soon as its out ops finish
                        eng[si % 2].dma_start(
                            out=o_d[img, cc * 128:(cc + 1) * 128,
                                    r_lo * W:r_hi * W],
                            in_=o_t[:, r_lo:r_hi],
                        )
                        # drip-feed next image's V transform between groups
                        for f in vops[:2]:
                            f()
                        vops = vops[2:]
            for f in vops:
                f()
    nc.compile()
    _CACHE["nc"] = nc
    return nc


def make_in_maps(inp, kernel, bias):
    xpad = np.zeros((B, CIN, HP, W + 2), np.float32)
    xpad[:, :, 1:1 + H, 1:1 + W] = inp
    ev = xpad[:, :, :, 0::2]
    od = xpad[:, :, :, 1::2]
    x_par = np.stack([ev, od], axis=3).astype(np.float16)   # [B,CIN,58,2,29]
    x_dev = np.ascontiguousarray(
        x_par.reshape(B, 2, 128, XF))

    w = np.asarray(kernel, np.float64)                      # [cout,cin,kh,kw]
    G = np.array([[1, 0, 0], [.5, .5, .5], [.5, -.5, .5], [0, 0, 1]], np.float64)
    Wt = np.einsum("pk,ochk->poch", G, w)                   # [4,cout,cin,kh]
    # SBUF tile layout is [cin128][p, kh, cinc, cout]; match it in DRAM so a
    # single DMA fills the tile.  cin = cinc*128 + cin128.
    w_dev = Wt.transpose(0, 3, 2, 1).reshape(4, K, 2, 128, 2, 128)  # p,kh,ci,cin,cc,co
    w_dev = np.ascontiguousarray(
        w_dev.transpose(4, 3, 0, 1, 2, 5)                 # cc,cin128,p,kh,ci,co
        .reshape(2, 128, 4 * K * 2 * 128).astype(np.float16))
    b_dev = np.ascontiguousarray(
        np.asarray(bias, np.float32).reshape(2, 128).T)     # [128, cc]
    return [
        {"x": np.ascontiguousarray(x_dev[c * BPC:(c + 1) * BPC]),
         "w": w_dev, "b": b_dev}
        for c in range(NCORES)
    ]


def assemble(results):
    o = np.concatenate([results[c]["o"] for c in range(NCORES)], axis=0)
    # device layout [.., h, par, tx] -> [.., h, 2tx+par]
    o = o.reshape(B, COUT, H, 2, NT).transpose(0, 1, 2, 4, 3)
    return np.ascontiguousarray(o.reshape(B, COUT, H, W).astype(np.float32))


def kernel(inp, kernel, bias):
    nc = _build()
    in_maps = make_in_maps(inp, kernel, bias)
    r = run_bass_kernel_spmd(nc, in_maps, core_ids=list(range(NCORES)))
    return assemble(r.results)
